# revision 3
# baseline (speedup 1.0000x reference)
"""Trainium2 Bass kernel for nn_LocalDenseCrossReadout.

Data-parallel over batch, one batch per core, no collectives.

Host-side folds (cheap numpy, <0.5% of module FLOPs):
- FiLM conditioning -> gamma/beta; LayerNorm of q/source (elementwise,
  exactly the reference f32 math) with affine+FiLM+scale folded into the
  projection weights; pre-transposed copies of the normalized
  activations so the device needs no transposes of z.
- scores = (z_q @ Wqk) @ z_k^T with Wqk = WqS @ WkS^T; k_p/v_p never
  materialized (banded attention covers only ~1.4x of kv, so folding
  W_v@W_o and W_k into the q side removes the dominant projections).
- readout out = (P_norm @ z_k) @ Wvo + bo' (softmax rows sum to 1 so
  the v-bias folds into bo').
- gate bias + mask fold into one reciprocal:
    P = exp(sc) / (exp(-gl) + mskv),  mskv = 1 valid / 1e30 masked.

Device kernel (per core), bf16 matmul datapath / f32 PSUM:
- Exp is the only table-based scalar function -> 1 ACT_TABLE_LOAD.
- P^T / oa^T via DMA-engine xbar transposes; PE does only matmuls.
- softmax without max-subtraction; row-sum fused into the P=P0*r
  multiply (accum_out); normalization folded into the readout copy
  (activation scale=rinv); 1/x via single-pass reciprocal_approx_fast.
- banded attention: per 128-row q tile a 640/768-wide aligned window.
"""

import sys

sys.path.insert(0, "/opt/trn_rl_repo")

import numpy as np
import ml_dtypes

import concourse.bass as bass
import concourse.tile as tile
from concourse import bacc
from concourse import mybir
from concourse.bass_utils import run_bass_kernel_spmd

BF16 = ml_dtypes.bfloat16
DIM, QS, QT, KS, KT, WIN, B, RANK = 512, 64, 16, 256, 16, 4, 8, 32
Q = QS * QT  # 1024
K = KS * KT  # 4096
NQT = Q // 128  # 8 q tiles
NKT = K // 128  # 32 kv blocks
F32 = mybir.dt.float32
BF = mybir.dt.bfloat16
FT = mybir.ActivationFunctionType
ALU = mybir.AluOpType

WSTARTS = [0, 384, 896, 1408, 1920, 2432, 2944, 3456]
NCH = [5, 6, 6, 6, 6, 6, 6, 5]


def build_bass():
    nc = bacc.Bacc("TRN2", target_bir_lowering=False)
    zq_d = nc.dram_tensor("zq", [Q, DIM], BF, kind="ExternalInput")
    zqt_d = nc.dram_tensor("zqt", [4, 128, Q], BF, kind="ExternalInput")
    zs_d = nc.dram_tensor("zs", [K, DIM], BF, kind="ExternalInput")
    zst_d = nc.dram_tensor("zst", [4, 128, K], BF, kind="ExternalInput")
    wqk_d = nc.dram_tensor("wqk", [DIM, DIM], BF, kind="ExternalInput")
    wvo_d = nc.dram_tensor("wvo", [DIM, DIM], BF, kind="ExternalInput")
    wqg_d = nc.dram_tensor("wqg", [DIM, RANK], BF, kind="ExternalInput")
    wkg_d = nc.dram_tensor("wkg", [DIM, RANK], BF, kind="ExternalInput")
    # consts [128, 10] f32: cols 0-3 r_qk (m-chunks), 4-7 bo' (m-chunks),
    # col 8 rows 0-31 r_qg, col 9 rows 0-31 r_kg
    cst_d = nc.dram_tensor("cst", [128, 10], F32, kind="ExternalInput")
    # reciprocal-denominator mask: 1 valid, 1e30 masked
    msk_d = nc.dram_tensor("msk", [NQT, 128, 768], BF, kind="ExternalInput")
    # out^T [m-chunk, part, q] ; host un-transposes + upcasts
    out_d = nc.dram_tensor("out", [4, 128, Q], BF, kind="ExternalOutput")

    with tile.TileContext(nc) as tc:
        with (
            tc.tile_pool(name="consts", bufs=1) as consts,
            tc.tile_pool(name="wts", bufs=1) as wts,
            tc.tile_pool(name="big", bufs=1) as big,
            tc.tile_pool(name="attn", bufs=3) as attn,
            tc.tile_pool(name="stats", bufs=4) as stats,
            tc.tile_pool(name="outp", bufs=2) as outp,
            tc.tile_pool(name="ps_b", bufs=3, space="PSUM") as ps_b,
            tc.tile_pool(name="ps_s", bufs=2, space="PSUM") as ps_s,
        ):
            # ---------------- inputs ----------------
            cst = consts.tile([128, 10], F32)
            nc.sync.dma_start(out=cst, in_=cst_d[:, :])
            warm = consts.tile([128, 512], BF)
            nc.vector.memset(warm, 0.5)
            msk_sb = big.tile([128, NQT, 768], BF, tag="msk")
            nc.gpsimd.dma_start(out=msk_sb, in_=msk_d.rearrange("t p w -> p t w"))

            def load_w(name, dram, n_out):
                t = wts.tile([128, 4, n_out], BF, tag=name)
                for c in range(4):
                    nc.sync.dma_start(out=t[:, c, :], in_=dram[c * 128:(c + 1) * 128, :])
                return t

            wqk_sb = load_w("wqk", wqk_d, DIM)
            wvo_sb = load_w("wvo", wvo_d, DIM)
            wqg_sb = load_w("wqg", wqg_d, RANK)
            wkg_sb = load_w("wkg", wkg_d, RANK)

            zTq = big.tile([128, NQT, 4, 128], BF, tag="zTq")
            for cz in range(4):
                nc.scalar.dma_start(
                    out=zTq[:, :, cz, :],
                    in_=zqt_d[cz].rearrange("p (b c) -> p b c", c=128))
            zn = big.tile([128, NKT, DIM], BF, tag="zn")
            for h in range(2):
                nc.gpsimd.dma_start(
                    out=zn[:, h * 16:(h + 1) * 16, :],
                    in_=zs_d[h * 2048:(h + 1) * 2048, :].rearrange(
                        "(j p) d -> p j d", j=16))
            zTk = big.tile([128, NKT, 4, 128], BF, tag="zTk")
            for cz in range(4):
                nc.scalar.dma_start(
                    out=zTk[:, :, cz, :],
                    in_=zst_d[cz].rearrange("p (b c) -> p b c", c=128))

            kqT = big.tile([128, 4, Q], BF, tag="kqT")       # (z_q@Wqk)^T
            gqT = big.tile([32, Q], BF, tag="gqT")
            gkT = big.tile([32, K], BF, tag="gkT")

            # PE warm-up: ramp the HAM clock while DMAs land
            for wi in range(12):
                wp = ps_s.tile([128, 512], F32, tag="ps", name="wp")
                nc.tensor.matmul(wp, warm[:, :128], warm, start=True, stop=True)

            # ---------------- projections ----------------
            # kq^T = Wqk^T-chunks @ zTq (+ r_qk bias)
            for m in range(4):
                for h in range(2):
                    pq = ps_s.tile([128, 512], F32, tag="ps", name="pq")
                    for c in range(4):
                        nc.tensor.matmul(
                            pq, wqk_sb[:, c, m * 128:(m + 1) * 128],
                            zTq[:, h * 4:(h + 1) * 4, c, :],
                            start=(c == 0), stop=(c == 3))
                    nc.scalar.activation(
                        out=kqT[:, m, h * 512:(h + 1) * 512], in_=pq,
                        func=FT.Identity, bias=cst[:, m:m + 1])
            # gq^T
            for h in range(2):
                gp_t = ps_s.tile([128, 512], F32, tag="ps", name="gp")
                gp = gp_t[0:32, :]
                for c in range(4):
                    nc.tensor.matmul(
                        gp, wqg_sb[:, c, :], zTq[:, h * 4:(h + 1) * 4, c, :],
                        start=(c == 0), stop=(c == 3))
                nc.scalar.activation(
                    out=gqT[:, h * 512:(h + 1) * 512], in_=gp,
                    func=FT.Identity, bias=cst[0:32, 8:9])
            # gk^T per supertile
            for sup in range(8):
                gp_t = ps_s.tile([128, 512], F32, tag="ps", name="gpk")
                gp = gp_t[0:32, :]
                for c in range(4):
                    nc.tensor.matmul(
                        gp, wkg_sb[:, c, :],
                        zTk[:, sup * 4:(sup + 1) * 4, c, :],
                        start=(c == 0), stop=(c == 3))
                nc.scalar.activation(
                    out=gkT[:, sup * 512:(sup + 1) * 512], in_=gp,
                    func=FT.Identity, bias=cst[0:32, 9:10])

            # ---------------- attention ----------------
            oaT4_cur = [None]

            def attn_tile(t):
                if t % 4 == 0:
                    oaT4_cur[0] = outp.tile([128, 4, 4, 128], BF, tag="oaT4",
                                            name="oaT4")
                oaT4 = oaT4_cur[0]
                w0, nch = WSTARTS[t], NCH[t]
                b0 = w0 // 128
                win = nch * 128
                qc = bass.ts(t, 128)
                # gate logits
                gl = ps_b.tile([128, 768], F32, tag="ps_big", name="gl")
                for n0 in (0, 512):
                    nn = min(512, win - n0)
                    nc.tensor.matmul(gl[:, n0:n0 + nn], gqT[:, qc],
                                     gkT[:, w0 + n0:w0 + n0 + nn],
                                     start=True, stop=True)
                u1 = attn.tile([128, 768], BF, tag="u1", name="u1")
                nc.scalar.activation(out=u1[:, :win], in_=gl[:, :win],
                                     func=FT.Exp, scale=-1.0)
                # r = 1/(exp(-gl) + mskv)  (= sigmoid * mask01)
                vv = attn.tile([128, 768], F32, tag="vv", name="vv")
                nc.vector.tensor_tensor(out=vv[:, :win], in0=u1[:, :win],
                                        in1=msk_sb[:, t, :win], op=ALU.add)
                rr = attn.tile([128, 768], F32, tag="rr", name="rr")
                nc.vector.reciprocal_approx_fast(out=rr[:, :win], in_=vv[:, :win])
                # scores
                sc = ps_b.tile([128, 768], F32, tag="ps_big", name="sc")
                for n0 in (0, 512):
                    nn = min(512, win - n0)
                    cb0 = b0 + n0 // 128
                    for c in range(4):
                        nc.tensor.matmul(sc[:, n0:n0 + nn], kqT[:, c, qc],
                                         zTk[:, cb0:cb0 + nn // 128, c, :],
                                         start=(c == 0), stop=(c == 3))
                P0 = attn.tile([128, 768], BF, tag="P0", name="P0")
                nc.scalar.activation(out=P0[:, :win], in_=sc[:, :win],
                                     func=FT.Exp)
                P = attn.tile([128, 768], BF, tag="P", name="P")
                rsum = stats.tile([128, 1], F32, tag="rsum", name="rsum")
                nc.vector.scalar_tensor_tensor(
                    out=P[:, :win], in0=P0[:, :win], scalar=1.0,
                    in1=rr[:, :win], op0=ALU.mult, op1=ALU.mult,
                    accum_out=rsum)
                rinv = stats.tile([128, 1], F32, tag="rinv", name="rinv")
                nc.vector.reciprocal_approx_fast(out=rinv, in_=rsum)
                # P^T via DMA xbar transpose
                PT = attn.tile([128, 6, 128], BF, tag="PT", name="PT")
                nc.sync.dma_start(out=PT[:, :nch, :], in_=P[:, :win],
                                  transpose=True)
                # readout: av = P @ z ; oa = av * rinv (scale in copy)
                av = ps_s.tile([128, 512], F32, tag="ps", name="av")
                for cc in range(nch):
                    nc.tensor.matmul(av, PT[:, cc, :], zn[:, b0 + cc, :],
                                     start=(cc == 0), stop=(cc == nch - 1))
                oa = outp.tile([128, DIM], BF, tag="oa", name="oa")
                nc.scalar.activation(out=oa, in_=av, func=FT.Identity,
                                     scale=rinv)
                nc.sync.dma_start(out=oaT4[:, t % 4, :, :], in_=oa,
                                  transpose=True)

            def out_batch(g):  # g = 0 or 1: q tiles 4g..4g+3
                oaT4 = oaT4_cur[0]
                ostage = outp.tile([128, 4, 512], BF, tag="ostage", name="ostage")
                for m in range(4):
                    po = ps_s.tile([128, 512], F32, tag="ps", name="po")
                    for c in range(4):
                        nc.tensor.matmul(
                            po, wvo_sb[:, c, m * 128:(m + 1) * 128],
                            oaT4[:, :, c, :], start=(c == 0), stop=(c == 3))
                    nc.scalar.activation(out=ostage[:, m, :], in_=po,
                                         func=FT.Identity,
                                         bias=cst[:, 4 + m:5 + m])
                nc.sync.dma_start(
                    out=out_d[:, :, g * 512:(g + 1) * 512].rearrange(
                        "m p q -> p m q"),
                    in_=ostage)

            for t in range(NQT):
                attn_tile(t)
                if t == 3:
                    out_batch(0)
            out_batch(1)

    if not nc.is_finalized():
        nc.finalize()
    return nc


_NC_CACHE = None


def _get_nc():
    global _NC_CACHE
    if _NC_CACHE is None:
        _NC_CACHE = build_bass()
    return _NC_CACHE


def _host_fold(inputs):
    f32 = np.float32
    scale = f32(DIM ** -0.5)
    tb = lambda a: np.ascontiguousarray(np.asarray(a, f32)).astype(BF16)
    ctx0 = np.asarray(inputs["ctx0"], f32)
    ctx1 = np.asarray(inputs["ctx1"], f32)
    pre = ctx0 @ inputs["Wc0"] + inputs["bc0"] + ctx1 @ inputs["Wc1"] + inputs["bc1"]
    pre = np.asarray(pre, f32)
    h = pre / (1.0 + np.exp(-pre))
    gb = np.asarray(h @ inputs["Wf"] + inputs["bf"], f32)
    gamma, beta = gb[:, :DIM], gb[:, DIM:]

    qn_g = np.asarray(inputs["qn_g"], f32)
    qn_b = np.asarray(inputs["qn_b"], f32)
    kvn_g = np.asarray(inputs["kvn_g"], f32)
    kvn_b = np.asarray(inputs["kvn_b"], f32)
    Wq, bq = np.asarray(inputs["Wq"], f32), np.asarray(inputs["bq"], f32)
    Wk, bk = np.asarray(inputs["Wk"], f32), np.asarray(inputs["bk"], f32)
    Wv, bv = np.asarray(inputs["Wv"], f32), np.asarray(inputs["bv"], f32)
    Wo, bo = np.asarray(inputs["Wo"], f32), np.asarray(inputs["bo"], f32)
    Wgq, Wgk = np.asarray(inputs["Wgq"], f32), np.asarray(inputs["Wgk"], f32)
    mask = np.asarray(inputs["mask"], f32)

    WkS = Wk * kvn_g[:, None]
    r_k = kvn_b @ Wk + bk
    WvS = Wv * kvn_g[:, None]
    r_v = kvn_b @ Wv + bv
    Wvo = tb(WvS @ Wo)
    bo_p = r_v @ Wo + bo
    Wkg = tb(WkS @ Wgk)
    r_kg = r_k @ Wgk
    WgqS = Wgq / scale / np.sqrt(f32(RANK))

    # reciprocal-denominator mask: 1 on valid, 1e30 on masked
    m01 = np.where(mask == 0.0, np.float32(1.0), np.float32(1e30))
    bmask = np.stack([
        m01[t * 128:(t + 1) * 128, w:w + 768] if w + 768 <= K else
        np.pad(m01[t * 128:(t + 1) * 128, w:K],
               ((0, 0), (0, w + 768 - K)), constant_values=1e30)
        for t, w in enumerate(WSTARTS)]).astype(BF16)

    query = np.asarray(inputs["query"], f32).reshape(B, Q, DIM)
    source = np.asarray(inputs["source"], f32).reshape(B, K, DIM)

    def ln(x):  # row-wise LayerNorm, exactly the reference f32 math
        mu = x.mean(-1, keepdims=True, dtype=np.float32)
        xc = x - mu
        var = np.mean(xc * xc, axis=-1, keepdims=True, dtype=np.float32)
        return xc / np.sqrt(var + np.float32(1e-5))

    zq_all = ln(query.reshape(B * Q, DIM)).reshape(B, Q, DIM).astype(BF16)
    zs_all = ln(source.reshape(B * K, DIM)).reshape(B, K, DIM).astype(BF16)

    in_maps = []
    for b in range(B):
        sg = qn_g * (1.0 + gamma[b])
        WqS = Wq * sg[:, None] * scale
        r_q = ((qn_b * (1.0 + gamma[b]) + beta[b]) @ Wq + bq) * scale
        Wqk = WqS @ WkS.T
        r_qk = r_q @ WkS.T
        Wqg = WqS @ WgqS
        r_qg = r_q @ WgqS
        cst = np.zeros((128, 10), f32)
        cst[:, 0:4] = r_qk.reshape(4, 128).T
        cst[:, 4:8] = bo_p.reshape(4, 128).T
        cst[0:32, 8] = r_qg
        cst[0:32, 9] = r_kg
        zq = zq_all[b]
        zs = zs_all[b]
        in_maps.append({
            "zq": zq,
            "zqt": np.ascontiguousarray(zq.T).reshape(4, 128, Q),
            "zs": zs,
            "zst": np.ascontiguousarray(zs.T).reshape(4, 128, K),
            "wqk": tb(Wqk), "wvo": Wvo,
            "wqg": tb(Wqg), "wkg": Wkg,
            "cst": cst,
            "msk": bmask,
        })
    return in_maps


def kernel(**inputs):
    nc = _get_nc()
    in_maps = _host_fold(inputs)
    res = run_bass_kernel_spmd(nc, in_maps, core_ids=list(range(B)))
    out = np.stack([
        np.ascontiguousarray(
            res.results[b]["out"].astype(np.float32).reshape(DIM, Q).T)
        for b in range(B)])
    return out.reshape(B, QS, QT, DIM)


if __name__ == "__main__":
    build_bass()
    print("bass build OK")
